# revision 10
# baseline (speedup 1.0000x reference)
"""Trainium2 Bass kernel for nn_BaseMoEModel (2-layer MoE transformer LM).

Sharding over 8 NeuronCores:
  - Attention: head-parallel (16 heads -> 2 heads/core); partial ao output
    summed with AllReduce.
  - MoE FFN: expert-parallel (8 experts -> 1 expert/core), dense all-token
    compute per expert weighted by top-2 gate combine weights; partial
    outputs summed with AllReduce.
  - LM head: vocab-parallel (32000 -> 4000 cols/core), host concat.

Precision: the top-2 routing decision is discrete, and the data has gate-prob
ties as small as 4e-5, so everything feeding the (last) gate runs in f32 on
the PE (4 cycles/row): attention both layers, layer-0 MoE, gates. Only the
layer-1 MoE value path and the LM head run in bf16 (their errors never feed
a routing decision). LN affines are folded into weights host-side; biases in
this model are zero (generic paths emitted only when nonzero).
"""
import sys
if "/opt/trn_rl_repo" not in sys.path:
    sys.path.insert(0, "/opt/trn_rl_repo")

from contextlib import ExitStack
import numpy as np

import concourse.bass as bass
import concourse.bacc as bacc
import concourse.tile as tile
from concourse import mybir
from concourse.bass_utils import run_bass_kernel_spmd

import ml_dtypes
BF16 = ml_dtypes.bfloat16

V, D, L, H, F, E, K2, MAXS = 32000, 1024, 2, 16, 4096, 8, 2, 2048
B, S = 2, 1024
T = B * S
P = 128
NCORES = 8
HLOC = H // NCORES            # 2
HD = D // H                   # 64
VLOC = V // NCORES            # 4000
NT = T // P                   # 16
ND = D // P                   # 8
NF = F // P                   # 32
TCH = 512
NTCH = T // TCH               # 4
TCH0 = 256                    # layer-0 moe token chunk (f32 hT fits SBUF)
NTCH0 = T // TCH0             # 8
NQJ = S // TCH                # 2
LBW = 0.01
EPS = 1e-5

f32 = mybir.dt.float32
bf16 = mybir.dt.bfloat16
AX = mybir.AxisListType.X
AF = mybir.ActivationFunctionType

REPLICA_GROUPS = [list(range(NCORES))]


def build_nc(nonzero=None, debug=False):
    nz = nonzero or {}
    nc = bacc.Bacc("TRN2", target_bir_lowering=False)
    dbg_out = None
    if debug:
        dbg_out = nc.dram_tensor("dbg", [4, NT, P, D], f32, kind="ExternalOutput")

    # ---- I/O ----
    x0_in = nc.dram_tensor("x0", [NT, P, D], f32, kind="ExternalInput")
    tri_in = nc.dram_tensor("tri", [P, P], f32, kind="ExternalInput")
    iden_in = nc.dram_tensor("iden", [P, P], f32, kind="ExternalInput")
    wqk_in = nc.dram_tensor("wqk", [L, P, ND, 2, P], f32, kind="ExternalInput")
    wv_in = nc.dram_tensor("wv", [L, P, ND, P], f32, kind="ExternalInput")
    wao_in = nc.dram_tensor("wao", [L, P, D], f32, kind="ExternalInput")
    wg_in = nc.dram_tensor("wg", [L, P, ND, E], f32, kind="ExternalInput")
    w1f_in = nc.dram_tensor("w1f", [NF, P, ND, P], f32, kind="ExternalInput")
    w1b_in = nc.dram_tensor("w1b", [NF, P, ND, P], bf16, kind="ExternalInput")
    b1_in = nc.dram_tensor("b1", [L, P, NF], f32, kind="ExternalInput")
    w2f_in = nc.dram_tensor("w2f", [2, 4, P, 8, TCH], f32, kind="ExternalInput")
    w2b_in = nc.dram_tensor("w2b", [2, 2, P, 16, TCH], bf16, kind="ExternalInput")
    wlm_in = nc.dram_tensor("wlm", [8, P, ND, 500], bf16, kind="ExternalInput")
    logits_out = nc.dram_tensor("logits", [NT, P, VLOC], f32, kind="ExternalOutput")
    probs_out = nc.dram_tensor("probs", [L, NT, P, E], f32, kind="ExternalOutput")
    if nz.get("b2"):
        b2b_in = nc.dram_tensor("b2b", [L, P, D], f32, kind="ExternalInput")
    if nz.get("aob"):
        aob_in = nc.dram_tensor("aob", [L, 1, D], f32, kind="ExternalInput")
    if nz.get("qkvb"):
        qkb_in = nc.dram_tensor("qkb", [L, 2, 1, P], f32, kind="ExternalInput")
        vb_in = nc.dram_tensor("vb", [L, 1, P], f32, kind="ExternalInput")
    if nz.get("gb"):
        gb_in = nc.dram_tensor("gb", [L, 1, E], f32, kind="ExternalInput")
    if nz.get("lmb"):
        lmb_in = nc.dram_tensor("lmb", [8, 1, 500], bf16, kind="ExternalInput")

    with tile.TileContext(nc) as tc, ExitStack() as ctx:
        persist = ctx.enter_context(tc.tile_pool(name="persist", bufs=1))
        sh16 = ctx.enter_context(tc.tile_pool(name="sh16", bufs=4))
        w1p = ctx.enter_context(tc.tile_pool(name="w1p", bufs=2))
        scrA = ctx.enter_context(tc.tile_pool(name="scrA", bufs=2))
        scrB = ctx.enter_context(tc.tile_pool(name="scrB", bufs=2))
        ppool = ctx.enter_context(tc.tile_pool(name="ppool", bufs=1))
        small = ctx.enter_context(tc.tile_pool(name="small", bufs=4))
        psum = ctx.enter_context(tc.tile_pool(name="psum", bufs=4, space="PSUM"))
        psaux = ctx.enter_context(tc.tile_pool(name="psaux", bufs=1, space="PSUM"))
        dram = ctx.enter_context(tc.tile_pool(name="dram", bufs=1, space="DRAM"))

        # ---- persistent SBUF ----
        nT = persist.tile([P, ND, T], f32)            # LN output, d-major, 64KB/p
        comb_sb = persist.tile([P, NT, E], f32)
        tri_sb = persist.tile([P, P], f32)
        iden_sb = persist.tile([P, P], f32)
        ones_col = persist.tile([P, 1], f32)
        ones_row_f = persist.tile([1, P], f32)
        ones_row_b = persist.tile([1, TCH], f32)
        b1_sb = persist.tile([P, NF], f32)

        x_d = dram.tile([NT, P, D], f32, tag="x_d")   # residual stream in DRAM

        nc.sync.dma_start(out=tri_sb[:], in_=tri_in[:, :])
        nc.sync.dma_start(out=iden_sb[:], in_=iden_in[:, :])
        nc.vector.memset(ones_col[:], 1.0)
        nc.vector.memset(ones_row_f[:], 1.0)
        nc.vector.memset(ones_row_b[:], 1.0)
        nc.sync.dma_start(out=x_d[:, :, :], in_=x0_in[:, :, :])

        def ln_tile_to_nT(xt, t):
            """LN of an SBUF token tile xt [128, D] f32 -> nT columns (PE transpose)."""
            ssum = small.tile([P, 1], f32, tag="ln_s")
            nc.vector.reduce_sum(ssum, xt, axis=AX)
            negmean = small.tile([P, 1], f32, tag="ln_nm")
            nc.vector.tensor_scalar_mul(negmean, ssum, -1.0 / D)
            sq = scrA.tile([P, D], f32, tag="scrA")
            ssq = small.tile([P, 1], f32, tag="ln_ssq")
            nc.scalar.activation(sq, xt, AF.Square, bias=negmean, scale=1.0,
                                 accum_out=ssq)
            veps = small.tile([P, 1], f32, tag="ln_veps")
            nc.vector.tensor_scalar(veps, ssq, 1.0 / D, EPS,
                                    op0=mybir.AluOpType.mult,
                                    op1=mybir.AluOpType.add)
            sd = small.tile([P, 1], f32, tag="ln_sd")
            nc.scalar.sqrt(sd, veps)
            rstd = small.tile([P, 1], f32, tag="ln_rstd")
            nc.vector.reciprocal(rstd, sd)
            nmr = small.tile([P, 1], f32, tag="ln_nmr")
            nc.vector.tensor_mul(nmr, negmean, rstd)
            nrm = scrB.tile([P, D], f32, tag="scrB")
            nc.scalar.activation(nrm, xt, AF.Identity, bias=nmr, scale=rstd)
            for dt_ in range(ND):
                pt = psaux.tile([P, P], f32, tag="ps_tr")
                nc.tensor.transpose(pt[:], nrm[:, dt_ * P:(dt_ + 1) * P],
                                    iden_sb[:])
                nc.scalar.copy(nT[:, dt_, t * P:(t + 1) * P], pt[:])

        def ln_from_dram():
            for t in range(NT):
                xt = scrA.tile([P, D], f32, tag="scrA")
                nc.sync.dma_start(out=xt[:], in_=x_d[t])
                ln_tile_to_nT(xt, t)

        def add_from_dram_and_ln(addend, do_ln=True, dbg_slot=None):
            for t in range(NT):
                xt = scrA.tile([P, D], f32, tag="scrA")
                nc.sync.dma_start(out=xt[:], in_=x_d[t])
                at = scrA.tile([P, D], f32, tag="scrA")
                nc.sync.dma_start(out=at[:], in_=addend[t])
                nc.vector.tensor_add(xt, xt, at)
                nc.sync.dma_start(out=x_d[t], in_=xt[:])
                if dbg_slot is not None and dbg_out is not None:
                    nc.sync.dma_start(out=dbg_out[dbg_slot, t], in_=xt[:])
                if do_ln:
                    ln_tile_to_nT(xt, t)

        for l in range(L):
            # ======== LN1 -> nT (layer 0 reads x_d fresh; later fused) ========
            if l == 0:
                ln_from_dram()

            # ======== q/k (feature-major), f32 ========
            wqk_sb = sh16.tile([P, ND, 2, P], f32, tag="sh16")
            nc.sync.dma_start(out=wqk_sb[:], in_=wqk_in[l])
            qk_sb = sh16.tile([P, 2, T], f32, tag="sh16")
            for m in range(2):
                for cch in range(T // TCH):
                    ps = psum.tile([P, TCH], f32, tag="ps512")
                    for kt in range(ND):
                        nc.tensor.matmul(
                            ps[:], wqk_sb[:, kt, m, :],
                            nT[:, kt, cch * TCH:(cch + 1) * TCH],
                            start=(kt == 0),
                            stop=(kt == ND - 1 and not nz.get("qkvb")))
                    if nz.get("qkvb"):
                        qkb_sb = small.tile([1, P], f32, tag="qkb")
                        nc.sync.dma_start(out=qkb_sb[:], in_=qkb_in[l, m])
                        nc.tensor.matmul(ps[:], qkb_sb[:], ones_row_b[:1, :],
                                         start=False, stop=True)
                    nc.scalar.copy(qk_sb[:, m, cch * TCH:(cch + 1) * TCH], ps[:])

            # ======== v (token-major), f32 ========
            wv_sb = sh16.tile([P, ND, P], f32, tag="sh16")
            nc.sync.dma_start(out=wv_sb[:], in_=wv_in[l])
            v_sb = sh16.tile([P, NT, P], f32, tag="sh16")
            for t in range(NT):
                ps = psum.tile([P, P], f32, tag="ps512")
                for kt in range(ND):
                    nc.tensor.matmul(
                        ps[:], nT[:, kt, t * P:(t + 1) * P], wv_sb[:, kt, :],
                        start=(kt == 0),
                        stop=(kt == ND - 1 and not nz.get("qkvb")))
                if nz.get("qkvb"):
                    vb_sb = small.tile([1, P], f32, tag="vb")
                    nc.sync.dma_start(out=vb_sb[:], in_=vb_in[l])
                    nc.tensor.matmul(ps[:], ones_row_b[:1, :P], vb_sb[:],
                                     start=False, stop=True)
                nc.scalar.copy(v_sb[:, t, :], ps[:])

            # ======== attention, f32 ========
            oT_sb = sh16.tile([P, T], f32, tag="sh16")
            for b in range(B):
                for h in range(HLOC):
                    q_h = qk_sb[h * HD:(h + 1) * HD, 0, b * S:(b + 1) * S]
                    k_h = qk_sb[h * HD:(h + 1) * HD, 1, b * S:(b + 1) * S]
                    for qj in range(NQJ):
                        kis = [ki for ki in range(ND) if (qj + 1) * TCH > ki * P]
                        expS = sh16.tile([P, ND, TCH], f32, tag="sh16")
                        for ki in range(ND):
                            if ki not in kis:
                                continue
                            ps = psum.tile([P, TCH], f32, tag="ps512")
                            nc.tensor.matmul(
                                ps[:], k_h[:, ki * P:(ki + 1) * P],
                                q_h[:, qj * TCH:(qj + 1) * TCH],
                                start=True, stop=True)
                            nc.scalar.activation(expS[:, ki, :], ps[:],
                                                 AF.Exp, scale=0.125)
                            dstart = ki * P - qj * TCH
                            if dstart >= 0:
                                if dstart > 0:
                                    nc.vector.memset(expS[:, ki, :dstart], 0.0)
                                nc.vector.tensor_mul(
                                    expS[:, ki, dstart:dstart + P],
                                    expS[:, ki, dstart:dstart + P], tri_sb[:])
                        psz = psaux.tile([1, TCH], f32, tag="ps_z")
                        for i, ki in enumerate(kis):
                            nc.tensor.matmul(
                                psz[:], ones_col[:], expS[:, ki, :],
                                start=(i == 0), stop=(i == len(kis) - 1))
                        zr = small.tile([1, TCH], f32, tag="zr")
                        nc.vector.reciprocal(zr, psz[:])
                        psb = psaux.tile([P, TCH], f32, tag="ps_zb")
                        nc.tensor.matmul(psb[:], ones_row_f[:], zr[:],
                                         start=True, stop=True)
                        zbq = scrB.tile([P, TCH], f32, tag="scrB")
                        nc.scalar.copy(zbq, psb[:])
                        pso = psum.tile([HD, TCH], f32, tag="ps512")
                        for i, ki in enumerate(kis):
                            nc.tensor.matmul(
                                pso[:], v_sb[:, b * ND + ki, h * HD:(h + 1) * HD],
                                expS[:, ki, :],
                                start=(i == 0), stop=(i == len(kis) - 1))
                        nc.vector.tensor_mul(
                            oT_sb[h * HD:(h + 1) * HD,
                                  b * S + qj * TCH:b * S + (qj + 1) * TCH],
                            pso[:], zbq[:HD, :])

            # ======== ao partial -> AllReduce -> x += (fused with LN2) ========
            wao_sb = sh16.tile([P, D], f32, tag="sh16")
            nc.sync.dma_start(out=wao_sb[:], in_=wao_in[l])
            ar_in = dram.tile([NT, P, D], f32, tag="ar_in")
            ar_out = dram.tile([NT, P, D], f32, tag="ar_out")
            for t in range(NT):
                otmp = scrA.tile([P, D], f32, tag="scrA")
                for cch in range(D // TCH):
                    ps = psum.tile([P, TCH], f32, tag="ps512")
                    nc.tensor.matmul(
                        ps[:], oT_sb[:, t * P:(t + 1) * P],
                        wao_sb[:, cch * TCH:(cch + 1) * TCH],
                        start=True, stop=not nz.get("aob"))
                    if nz.get("aob"):
                        aob_sb = small.tile([1, D], f32, tag="aob")
                        if t == 0 and cch == 0:
                            nc.sync.dma_start(out=aob_sb[:], in_=aob_in[l])
                        nc.tensor.matmul(
                            ps[:], ones_row_b[:1, :P],
                            aob_sb[:, cch * TCH:(cch + 1) * TCH],
                            start=False, stop=True)
                    nc.scalar.copy(otmp[:, cch * TCH:(cch + 1) * TCH], ps[:])
                nc.sync.dma_start(out=ar_in[t], in_=otmp[:])
            nc.gpsimd.collective_compute(
                "AllReduce", mybir.AluOpType.add,
                replica_groups=REPLICA_GROUPS,
                ins=[ar_in.opt()], outs=[ar_out.opt()])
            add_from_dram_and_ln(ar_out, do_ln=True,
                                 dbg_slot=(2 * l + 0) if debug else None)

            # ======== gate -> probs -> top-2 comb (col 0 = own expert) ========
            wg_sb = sh16.tile([P, ND, E], f32, tag="sh16")
            nc.sync.dma_start(out=wg_sb[:], in_=wg_in[l])
            nc.sync.dma_start(out=b1_sb[:], in_=b1_in[l])
            probs_sb = ppool.tile([P, NT, E], f32, tag="probs")
            for t in range(NT):
                psg = psaux.tile([P, E], f32, tag="ps_g")
                for kt in range(ND):
                    nc.tensor.matmul(
                        psg[:], nT[:, kt, t * P:(t + 1) * P], wg_sb[:, kt, :],
                        start=(kt == 0),
                        stop=(kt == ND - 1 and not nz.get("gb")))
                if nz.get("gb"):
                    gb_sb = small.tile([1, E], f32, tag="gb")
                    nc.sync.dma_start(out=gb_sb[:], in_=gb_in[l])
                    nc.tensor.matmul(psg[:], ones_row_b[:1, :P], gb_sb[:],
                                     start=False, stop=True)
                m = small.tile([P, 1], f32, tag="g_m")
                nc.vector.reduce_max(m, psg[:], axis=AX)
                negm = small.tile([P, 1], f32, tag="g_negm")
                nc.vector.tensor_scalar_mul(negm, m, -1.0)
                ex = small.tile([P, E], f32, tag="g_ex")
                se = small.tile([P, 1], f32, tag="g_se")
                nc.scalar.activation(ex, psg[:], AF.Exp, bias=negm, scale=1.0,
                                     accum_out=se)
                rz = small.tile([P, 1], f32, tag="g_rz")
                nc.vector.reciprocal(rz, se)
                nc.vector.tensor_scalar_mul(probs_sb[:, t, :], ex, rz)
                m1 = small.tile([P, 1], f32, tag="g_m1")
                nc.vector.reduce_max(m1, probs_sb[:, t, :], axis=AX)
                t0 = small.tile([P, E], f32, tag="g_t0")
                nc.vector.tensor_scalar(t0, probs_sb[:, t, :], m1, 2.0,
                                        op0=mybir.AluOpType.is_ge,
                                        op1=mybir.AluOpType.mult)
                msk = small.tile([P, E], f32, tag="g_msk")
                nc.vector.tensor_sub(msk, probs_sb[:, t, :], t0)
                m2 = small.tile([P, 1], f32, tag="g_m2")
                nc.vector.reduce_max(m2, msk, axis=AX)
                keep = small.tile([P, E], f32, tag="g_keep")
                nc.vector.tensor_scalar(keep, probs_sb[:, t, :], m2, None,
                                        op0=mybir.AluOpType.is_ge)
                s12 = small.tile([P, 1], f32, tag="g_s12")
                nc.vector.tensor_add(s12, m1, m2)
                rs = small.tile([P, 1], f32, tag="g_rs")
                nc.vector.reciprocal(rs, s12)
                pk = small.tile([P, E], f32, tag="g_pk")
                nc.vector.tensor_mul(pk, probs_sb[:, t, :], keep)
                nc.vector.tensor_scalar_mul(comb_sb[:, t, :], pk, rs)
            nc.sync.dma_start(out=probs_out[l].rearrange("t p e -> p t e"),
                              in_=probs_sb[:])

            # ======== MoE FFN partial -> AllReduce -> x += (fused next LN) ====
            ar2_in = dram.tile([NT, P, D], f32, tag="ar2_in")
            ar2_out = dram.tile([NT, P, D], f32, tag="ar2_out")
            if nz.get("b2"):
                b2b_sb = persist.tile([P, D], f32)
                nc.sync.dma_start(out=b2b_sb[:], in_=b2b_in[l])

            def y_evict(ps, tglob, n):
                ytmp = scrB.tile([P, TCH], f32, tag="scrB")
                nc.scalar.activation(ytmp[:, :TCH // 1], ps[:], AF.Copy,
                                     scale=comb_sb[:, tglob, 0:1])
                if nz.get("b2"):
                    b2s = scrB.tile([P, TCH], f32, tag="scrB")
                    nc.scalar.activation(
                        b2s, b2b_sb[:, n * TCH:(n + 1) * TCH], AF.Copy,
                        scale=comb_sb[:, tglob, 0:1])
                    nc.vector.tensor_add(ytmp, ytmp, b2s)
                nc.sync.dma_start(
                    out=ar2_in[tglob, :, n * TCH:(n + 1) * TCH], in_=ytmp)

            if l == 0:
                hT = persist.tile([P, NF, TCH0], f32, tag="hT")
                for c in range(NTCH0):
                    for mfb in range(NF):
                        w1t = w1p.tile([P, ND, P], f32, tag="w1s")
                        nc.sync.dma_start(out=w1t[:], in_=w1f_in[mfb])
                        ps = psum.tile([P, TCH0], f32, tag="ps512")
                        for kt in range(ND):
                            nc.tensor.matmul(
                                ps[:], w1t[:, kt, :],
                                nT[:, kt, c * TCH0:(c + 1) * TCH0],
                                start=(kt == 0), stop=(kt == ND - 1))
                        nc.scalar.activation(hT[:, mfb, :], ps[:], AF.Gelu,
                                             bias=b1_sb[:, mfb:mfb + 1],
                                             scale=1.0)
                    for n in range(2):
                        w2t = []
                        for gg in range(4):
                            w2g = sh16.tile([P, 8, TCH], f32, tag="sh16")
                            nc.sync.dma_start(out=w2g[:], in_=w2f_in[n, gg])
                            w2t.append(w2g)
                        for mt in range(TCH0 // P):
                            tglob = (c * TCH0 + mt * P) // P
                            ps = psum.tile([P, TCH], f32, tag="ps512")
                            for gg in range(4):
                                for k8 in range(8):
                                    kt = gg * 8 + k8
                                    nc.tensor.matmul(
                                        ps[:], hT[:, kt, mt * P:(mt + 1) * P],
                                        w2t[gg][:, k8, :],
                                        start=(kt == 0), stop=(kt == NF - 1))
                            y_evict(ps, tglob, n)
            else:
                hTb = persist.tile([P, NF, TCH], bf16, tag="hT")
                for c in range(NTCH):
                    n2b = sh16.tile([P, ND, TCH], bf16, tag="sh16")
                    nc.scalar.copy(n2b[:], nT[:, :, c * TCH:(c + 1) * TCH])
                    for mfb in range(NF):
                        w1t = w1p.tile([P, ND, P], bf16, tag="w1s")
                        nc.sync.dma_start(out=w1t[:], in_=w1b_in[mfb])
                        ps = psum.tile([P, TCH], f32, tag="ps512")
                        for kt in range(ND):
                            nc.tensor.matmul(
                                ps[:], w1t[:, kt, :], n2b[:, kt, :],
                                start=(kt == 0), stop=(kt == ND - 1))
                        nc.scalar.activation(hTb[:, mfb, :], ps[:], AF.Gelu,
                                             bias=b1_sb[:, mfb:mfb + 1],
                                             scale=1.0)
                    for n in range(2):
                        w2t = []
                        for G in range(2):
                            w2g = sh16.tile([P, 16, TCH], bf16, tag="sh16")
                            nc.sync.dma_start(out=w2g[:], in_=w2b_in[n, G])
                            w2t.append(w2g)
                        for mt in range(4):
                            tglob = c * 4 + mt
                            ps = psum.tile([P, TCH], f32, tag="ps512")
                            for G in range(2):
                                for k16 in range(16):
                                    kt = G * 16 + k16
                                    nc.tensor.matmul(
                                        ps[:], hTb[:, kt, mt * P:(mt + 1) * P],
                                        w2t[G][:, k16, :],
                                        start=(kt == 0), stop=(kt == NF - 1))
                            y_evict(ps, tglob, n)
            nc.gpsimd.collective_compute(
                "AllReduce", mybir.AluOpType.add,
                replica_groups=REPLICA_GROUPS,
                ins=[ar2_in.opt()], outs=[ar2_out.opt()])
            add_from_dram_and_ln(ar2_out, do_ln=True,
                                 dbg_slot=(2 * l + 1) if debug else None)

        # ======== LM head (vocab slice), bf16 ========
        for n in range(8):
            wlmt = sh16.tile([P, ND, 500], bf16, tag="sh16")
            nc.sync.dma_start(out=wlmt[:], in_=wlm_in[n])
            if nz.get("lmb"):
                lmb_sb = small.tile([1, 500], bf16, tag="lmb")
                nc.sync.dma_start(out=lmb_sb[:], in_=lmb_in[n])
            for t in range(NT):
                ntb = scrB.tile([P, ND, P], bf16, tag="ntb")
                nc.scalar.copy(ntb[:], nT[:, :, t * P:(t + 1) * P])
                ps = psum.tile([P, 500], f32, tag="ps512")
                for kt in range(ND):
                    nc.tensor.matmul(
                        ps[:], ntb[:, kt, :], wlmt[:, kt, :],
                        start=(kt == 0),
                        stop=(kt == ND - 1 and not nz.get("lmb")))
                if nz.get("lmb"):
                    onesb = small.tile([1, P], bf16, tag="onesb")
                    if n == 0 and t == 0:
                        nc.vector.memset(onesb[:], 1.0)
                    nc.tensor.matmul(ps[:], onesb[:], lmb_sb[:],
                                     start=False, stop=True)
                lg = scrB.tile([P, 500], f32, tag="lg")
                nc.scalar.copy(lg, ps[:])
                nc.sync.dma_start(out=logits_out[t, :, n * 500:(n + 1) * 500],
                                  in_=lg)

    nc.compile()
    return nc


# ---------------- host side ----------------

_NC_CACHE = {}


def _prep_inputs(inputs):
    g = {k: np.asarray(v) for k, v in inputs.items()}
    ids = g["input_ids"].astype(np.int64)
    tok_emb = g["tok_emb"].astype(np.float32)
    pos_emb = g["pos_emb"].astype(np.float32)

    x0 = (tok_emb[ids] + pos_emb[:S][None]).reshape(NT, P, D).astype(np.float32)
    tri = np.triu(np.ones((P, P), np.float32))  # keep key<=query
    iden = np.eye(P, dtype=np.float32)

    nz = {
        "qkvb": bool(np.any(g["qkv_b"])) or bool(np.any(g["ln1_b"])),
        "aob": bool(np.any(g["ao_b"])),
        "gb": bool(np.any(g["gate_b"])) or bool(np.any(g["ln2_b"])),
        "b2": bool(np.any(g["b2"])),
        "lmb": bool(np.any(g["fln_b"])),
    }

    in_maps = []
    for core in range(NCORES):
        m = {"x0": x0, "tri": tri, "iden": iden}
        wqk = np.zeros((L, P, ND, 2, P), np.float32)
        wv = np.zeros((L, P, ND, P), np.float32)
        wao = np.zeros((L, P, D), np.float32)
        wg = np.zeros((L, P, ND, E), np.float32)
        b1 = np.zeros((L, P, NF), np.float32)
        if nz["qkvb"]:
            qkb = np.zeros((L, 2, 1, P), np.float32)
            vb = np.zeros((L, 1, P), np.float32)
        if nz["aob"]:
            aob = np.zeros((L, 1, D), np.float32)
        if nz["gb"]:
            gb = np.zeros((L, 1, E), np.float32)
        if nz["b2"]:
            b2b = np.zeros((L, P, D), np.float32)
        h0 = core * HLOC
        e = core
        perm = [(e + j) % E for j in range(E)]
        for l in range(L):
            g1 = g["ln1_w"][l].astype(np.float32)
            be1 = g["ln1_b"][l].astype(np.float32)
            g2 = g["ln2_w"][l].astype(np.float32)
            be2 = g["ln2_b"][l].astype(np.float32)
            qkvw = g["qkv_w"][l].astype(np.float32)
            qcols = qkvw[:, h0 * HD:(h0 + HLOC) * HD]
            kcols = qkvw[:, D + h0 * HD:D + (h0 + HLOC) * HD]
            vcols = qkvw[:, 2 * D + h0 * HD:2 * D + (h0 + HLOC) * HD]
            wq = (g1[:, None] * qcols).reshape(ND, P, P)
            wk = (g1[:, None] * kcols).reshape(ND, P, P)
            wvl = (g1[:, None] * vcols).reshape(ND, P, P)
            wqk[l, :, :, 0, :] = wq.transpose(1, 0, 2)
            wqk[l, :, :, 1, :] = wk.transpose(1, 0, 2)
            wv[l] = wvl.transpose(1, 0, 2)
            wao[l] = g["ao_w"][l].astype(np.float32)[h0 * HD:(h0 + HLOC) * HD, :]
            gwl = (g2[:, None] * g["gate_w"][l].astype(np.float32))[:, perm]
            wg[l] = gwl.reshape(ND, P, E).transpose(1, 0, 2)
            w1fold = (g2[:, None] * g["w1"][l, e].astype(np.float32))
            w1l = w1fold.reshape(ND, P, NF, P).transpose(2, 1, 0, 3)
            w2full = g["w2"][l, e].astype(np.float32)
            if l == 0:
                m["w1f"] = np.ascontiguousarray(w1l, dtype=np.float32)
                m["w2f"] = np.ascontiguousarray(
                    w2full.reshape(4, 8, P, 2, TCH).transpose(3, 0, 2, 1, 4),
                    dtype=np.float32)
            else:
                m["w1b"] = w1l.astype(BF16)
                m["w2b"] = w2full.reshape(2, 16, P, 2, TCH).transpose(
                    3, 0, 2, 1, 4).astype(BF16)
            b1l = g["b1"][l, e].astype(np.float32) + be2 @ g["w1"][l, e].astype(np.float32)
            b1[l] = b1l.reshape(NF, P).T
            if nz["qkvb"]:
                qb = g["qkv_b"][l].astype(np.float32)
                qkb[l, 0, 0] = qb[h0 * HD:(h0 + HLOC) * HD] + be1 @ qcols
                qkb[l, 1, 0] = qb[D + h0 * HD:D + (h0 + HLOC) * HD] + be1 @ kcols
                vb[l, 0] = (qb[2 * D + h0 * HD:2 * D + (h0 + HLOC) * HD]
                            + be1 @ vcols)
            if nz["aob"] and core == 0:
                aob[l, 0] = g["ao_b"][l].astype(np.float32)
            if nz["gb"]:
                gb[l, 0] = (g["gate_b"][l].astype(np.float32)
                            + be2 @ g["gate_w"][l].astype(np.float32))[perm]
            if nz["b2"]:
                b2b[l] = np.tile(g["b2"][l, e].astype(np.float32), (P, 1))
        fg = g["fln_w"].astype(np.float32)
        wlm_full = (fg[:, None] * tok_emb.T)[:, core * VLOC:(core + 1) * VLOC]
        wlm = wlm_full.reshape(ND, P, 8, 500).transpose(2, 1, 0, 3).astype(BF16)
        m.update(wqk=wqk, wv=wv, wao=wao, wg=wg, b1=b1, wlm=wlm)
        if nz["qkvb"]:
            m.update(qkb=qkb, vb=vb)
        if nz["aob"]:
            m["aob"] = aob
        if nz["gb"]:
            m["gb"] = gb
        if nz["b2"]:
            m["b2b"] = b2b
        if nz["lmb"]:
            lmbf = g["fln_b"].astype(np.float32) @ tok_emb.T
            m["lmb"] = lmbf[core * VLOC:(core + 1) * VLOC].reshape(
                8, 1, 500).astype(BF16)
        in_maps.append(m)
    return in_maps, nz


def _lb_from_probs(probs):
    total = np.float32(0.0)
    for l in range(L):
        p = probs[l]
        am = p.argmax(-1)
        f = np.bincount(am, minlength=E).astype(np.float32) / p.shape[0]
        Pm = p.mean(0, dtype=np.float32)
        total += np.float32(LBW * E * (f * Pm).sum())
    return np.float32(total)


def kernel(**inputs):
    in_maps, nz = _prep_inputs(inputs)
    key = tuple(sorted(nz.items()))
    if key not in _NC_CACHE:
        _NC_CACHE[key] = build_nc(nonzero=nz)
    nc = _NC_CACHE[key]
    res = run_bass_kernel_spmd(nc, in_maps, core_ids=list(range(NCORES)))
    logits = np.concatenate(
        [res.results[c]["logits"].reshape(T, VLOC) for c in range(NCORES)],
        axis=1).reshape(B, S, V).astype(np.float32)
    probs = res.results[0]["probs"].reshape(L, T, E)
    lb = _lb_from_probs(probs)
    return logits, lb
